# revision 36
# baseline (speedup 1.0000x reference)
"""Trainium2 Bass kernel for the sketched-attention RS_SM op.

Reference semantics (per (b,h) pair):
    X  = concat([Q, K], axis=seq)                      # [4096, 64]
    XS = gather of 1024 landmark rows of X             # [m=4, d=256, 64]
    AS[n, d] = sum_m sign[m, d] * exp(X[n] . XS[m, d]) # [4096, 256]

Sharding: 16 (b,h) pairs over 8 cores = 2 pairs/core, no cross-core comms.

ALL matmuls run in the single 128x32 col-tiled PE mode (4 concurrent
32-wide tiles) -- mixing tiling modes forces a TensorE drain per switch,
which cost ~18us/run here.  Pipeline per (token-chunk t of 512, pair):
  MM1  : lhsT = landmarksT [128, 32] (zero-padded outside the pair's 64
         X-feature rows, so the full-contract matmul over the stacked
         [pair0;pair1] X^T drops the other pair), rhs = X^T [128, 512]
         -> PSUM [32 lmk, 512 n] per col tile, 4 tiles per chunk.
  exp  : split across two engines reading PSUM directly:
         - ScalarE activation Exp for chunks c0..c3 -> SBUF bf16
         - VectorE Schraudolph fast-exp for chunks c4..c7: one
           tensor_scalar (x*A + B) with int16 output whose bits ARE the
           bf16 of exp(x)  (A = 128/ln2, B = 127*128 - C).
  MM2  : lhsT = sign-delta W [128, 32], rhs = exp tile [128, 512]
         -> PSUM [32 d, 512 n], 4 col-tiles concurrent per half.
         This performs the signed reduction over m on the TensorE.
         MM2 is software-pipelined one step behind MM1 so the PE queue
         never blocks on an exp that hasn't finished.
  copy : PSUM -> SBUF bf16 [128, 512] per half, balanced between
         ScalarE and VectorE so both engines finish together.
  out  : two contiguous DMAs per pair: [128, 2, 2048] bf16.

Landmark order is permuted (host-side) so chunk c holds (m, dl) for
d = 32c + dl: partition p = 32*m + dl.  W[32m+dl, 32c+dl] = sign[m, 32c+dl].
Output HBM layout is [pair, 128 p, 2 h, 4096 n] bf16 with d = 128h + p;
the host decodes to [4096, 256] f32 at unshard.

All three device inputs (X^T | landmarks^T | W) are packed into one
[128, 5376] array so a single DMA (one semaphore lane) feeds the PE --
multiple DMA waits on one fused-LDW matmul overflow its sync-wait slots.

KERNEL_ITERS repeats the body (same in/out) inside one NEFF so the
benchmark can measure the marginal per-iteration hardware time,
excluding the multi-ms host->axon dispatch overhead.
"""

import math
import os
import sys
import types
from contextlib import ExitStack

import numpy as np

sys.path.insert(0, "/opt/trn_rl_repo")

# The axon client in this container lacks the NTFF profile hook module;
# provide a stub so bass_utils' trace path degrades gracefully.
try:
    import antenv.axon_hooks  # noqa: F401
except ImportError:
    _stub = types.ModuleType("antenv.axon_hooks")
    _stub.get_axon_ntff_profile_hook = lambda: None
    sys.modules["antenv.axon_hooks"] = _stub

import concourse.bacc as bacc
import concourse.bass as bass
import concourse.mybir as mybir
import concourse.tile as tile

B, H, N, P = 2, 8, 2048, 64
M, D = 4, 256
SEQ2 = 2 * N                      # 4096 tokens per pair
NCORES = 8
PAIRS = (B * H) // NCORES         # 2 pairs per core
L = M * D                         # 1024 landmarks per pair
TCH = 512                         # token chunk (matmul moving dim)
NT = SEQ2 // TCH                  # 8 token chunks
NC_ = 8                           # landmark chunks of 128
INW = SEQ2 + PAIRS * L + D        # packed input width: xt | lt0 | lt1 | w
F32 = mybir.dt.float32
BF16 = mybir.dt.bfloat16
I16 = mybir.dt.int16

# Schraudolph fast-exp in bf16-bit domain: bits16 = round(x*A + B);
# bitcast16 -> bf16 ~= exp(x).  C trades max vs rms error.
EXP_A = 128.0 / math.log(2.0)
EXP_C = float(os.environ.get("KERNEL_EXP_C", "5.0"))
EXP_B = 127.0 * 128.0 - EXP_C

# of the 4 exp groups (2 chunks each), how many run on VectorE via the
# approximate fast-exp (taken from the top: groups 3, 2, ...).
DVE_GROUPS = int(os.environ.get("KERNEL_DVE_GROUPS", "2"))

# timing-only bisection probes: "no_mm1" / "no_mm2" / "no_exp" / "no_dma"
PROBE = os.environ.get("KERNEL_PROBE", "")

_nc_cache = {}


def _copy_split(dve_groups: int) -> int:
    """# of the 32 PSUM->SBUF output copies to run on ScalarE so that
    ScalarE (exp + copies) and VectorE (exp + copies) finish together.
    Constants are CoreSim-calibrated per-instruction durations (ns)."""
    # HW-measured ScalarE overhead is ~352 cyc/instr (not the errata 172),
    # so weight the split with the (N+352)/1.2 and (N+151)/0.96 formulas.
    sc_exp = 16 * (4 - dve_groups) * 1147.0
    ve_exp = 16 * dve_groups * 1224.0
    # sc_exp + x*720 == ve_exp + (32-x)*691
    x = (ve_exp + 32 * 691.0 - sc_exp) / (720.0 + 691.0)
    return max(0, min(32, int(round(x))))


def _build_nc(iters: int = 1):
    nc = bacc.Bacc(
        "TRN2", target_bir_lowering=False, debug=False, num_devices=NCORES,
    )

    inp = nc.dram_tensor("inp", [128, INW], BF16, kind="ExternalInput")
    out = nc.dram_tensor("out", [PAIRS, 128, 2 * SEQ2], BF16,
                         kind="ExternalOutput")

    n_scalar_copies = _copy_split(DVE_GROUPS)

    with tile.TileContext(nc) as tc, ExitStack() as ctx:
        inp_pool = ctx.enter_context(tc.tile_pool(name="inp", bufs=2))
        eps_pool = ctx.enter_context(
            tc.tile_pool(name="eps", bufs=3, space="PSUM"))
        asps_pool = ctx.enter_context(
            tc.tile_pool(name="asps", bufs=2, space="PSUM"))
        esb_pool = ctx.enter_context(tc.tile_pool(name="esb", bufs=2))
        assb_pool = ctx.enter_context(tc.tile_pool(name="assb", bufs=2))

        def body():
            inp_sb = inp_pool.tile([128, INW], BF16, tag="inp")
            nc.sync.dma_start(inp_sb[:], inp[:])
            xt_sb = inp_sb[:, 0:SEQ2]
            lt_sb = [inp_sb[:, SEQ2 + L * p:SEQ2 + L * (p + 1)]
                     for p in range(PAIRS)]
            w_sb = inp_sb[:, SEQ2 + PAIRS * L:INW]

            # software pipeline over 16 steps s = (pr, t): step s emits
            # MM1+exp for s and MM2+copy for s-1, so the PE never sits in
            # its queue behind an MM2 whose exp hasn't finished.
            state = {}          # step -> (e_sb, as_sb, pr, t)
            as_tiles = {}

            def stage1(s):
                t, pr = divmod(s, PAIRS)
                if t == 0:
                    as_tiles[pr] = assb_pool.tile(
                        [128, 2 * SEQ2], BF16, tag="assb", name="as_sb")
                # full-contract rhs: both pairs' X stacked; the other
                # pair's rows in lhsT are zero, killing cross terms
                rhs_x = xt_sb[:, t * TCH:(t + 1) * TCH]
                e_sb = esb_pool.tile([128, NC_ * TCH], BF16, tag="esb")
                for g in range(4):
                    e_ps = eps_pool.tile([128, 2 * TCH], F32, tag="eps")
                    for gi in range(2):
                        c = 2 * g + gi
                        if PROBE == "no_mm1" and gi > 0:
                            continue
                        # uniform 128x32 PE tiling mode: 4 concurrent col
                        # tiles, one per 32-landmark (m) slice of chunk c;
                        # contract rows come from the operands' partitions
                        for k in range(4):
                            nc.tensor.matmul(
                                e_ps[32 * k:32 * (k + 1),
                                     gi * TCH:(gi + 1) * TCH],
                                lhsT=lt_sb[pr][:,
                                              128 * c + 32 * k:
                                              128 * c + 32 * (k + 1)],
                                rhs=rhs_x,
                                start=True, stop=True,
                                tile_position=(0, 32 * k),
                            )
                    dst = e_sb[:, 2 * g * TCH:2 * (g + 1) * TCH]
                    if PROBE == "no_exp":
                        continue
                    if g >= 4 - DVE_GROUPS:
                        # VectorE fast-exp: bf16 bits via int16 write
                        nc.vector.tensor_scalar(
                            dst.bitcast(I16),
                            e_ps[:],
                            EXP_A, EXP_B,
                            mybir.AluOpType.mult, mybir.AluOpType.add,
                        )
                    else:
                        nc.scalar.activation(
                            dst, e_ps[:],
                            mybir.ActivationFunctionType.Exp,
                        )
                state[s] = (e_sb, as_tiles[pr], pr, t)

            def stage2(s):
                e_sb, as_sb, pr, t = state.pop(s)
                as_hview = as_sb[:].rearrange("p (h n) -> p h n", h=2)
                for half in range(2):
                    as_ps = asps_pool.tile([128, TCH], F32, tag="asps",
                                           name="as_ps")
                    for j in range(4):
                        c = 4 * half + j
                        if PROBE == "no_mm2" and j > 0:
                            continue
                        nc.tensor.matmul(
                            as_ps[32 * j:32 * (j + 1), :],
                            lhsT=w_sb[:, 32 * c:32 * (c + 1)],
                            rhs=e_sb[:, c * TCH:(c + 1) * TCH],
                            start=True, stop=True,
                            tile_position=(0, 32 * j),
                        )
                    # copy PSUM -> SBUF bf16, engine-balanced
                    dst = as_hview[:, half, t * TCH:(t + 1) * TCH]
                    k = 2 * s + half
                    if (k * n_scalar_copies) // 32 != \
                            ((k + 1) * n_scalar_copies) // 32:
                        nc.scalar.copy(dst, as_ps[:])
                    else:
                        nc.vector.tensor_copy(dst, as_ps[:])
                # drain finished token-chunk ranges early: after t=3 and t=7
                if PROBE == "no_dma":
                    return
                if t == NT // 2 - 1:
                    nc.sync.dma_start(
                        out[pr].rearrange("p (h n) -> p h n", h=2)[
                            :, :, 0:(NT // 2) * TCH],
                        as_hview[:, :, 0:(NT // 2) * TCH])
                elif t == NT - 1:
                    nc.sync.dma_start(
                        out[pr].rearrange("p (h n) -> p h n", h=2)[
                            :, :, (NT // 2) * TCH:SEQ2],
                        as_hview[:, :, (NT // 2) * TCH:SEQ2])

            for s in range(PAIRS * NT):
                stage1(s)
                if s > 0:
                    stage2(s - 1)
            stage2(PAIRS * NT - 1)

        if iters == 1:
            body()
        else:
            # unroll 4 bodies per hardware-loop iteration so the per-
            # iteration all-engine barrier amortizes over 4 kernel runs
            assert iters % 4 == 0, iters
            with tc.For_i(0, iters // 4):
                for _u in range(4):
                    body()
    nc.compile()
    return nc


def _get_nc(iters: int = 1):
    key = (iters, DVE_GROUPS)
    if key not in _nc_cache:
        _nc_cache[key] = _build_nc(iters)
    return _nc_cache[key]


_runner_cache = {}


def _get_runner(iters: int = 1):
    """Build (once) a jitted shard_map callable over the 8 cores, mirroring
    bass2jax.run_bass_via_pjrt but cached so repeat calls don't re-trace."""
    key = (iters, DVE_GROUPS)
    if key in _runner_cache:
        return _runner_cache[key]
    import jax
    from jax.sharding import Mesh, PartitionSpec
    try:
        from jax.experimental.shard_map import shard_map
    except ImportError:
        from jax.shard_map import shard_map  # newer jax
    from concourse import bass2jax as b2j

    b2j.install_neuronx_cc_hook()
    nc = _get_nc(iters)

    partition_name = (
        nc.partition_id_tensor.name if nc.partition_id_tensor else None
    )
    in_names, out_names, out_avals, zero_shapes = [], [], [], []
    for alloc in nc.m.functions[0].allocations:
        if not isinstance(alloc, mybir.MemoryLocationSet):
            continue
        name = alloc.memorylocations[0].name
        if alloc.kind == "ExternalInput":
            if name != partition_name:
                in_names.append(name)
        elif alloc.kind == "ExternalOutput":
            out_names.append(name)
            shape = tuple(alloc.tensor_shape)
            dtype = mybir.dt.np(alloc.dtype)
            out_avals.append(jax.core.ShapedArray(shape, dtype))
            zero_shapes.append((shape, dtype))
    n_params = len(in_names)
    n_outs = len(out_avals)
    all_names = list(in_names) + list(out_names)
    if partition_name is not None:
        all_names.append(partition_name)
    donate = tuple(range(n_params, n_params + n_outs))

    def _body(*args):
        operands = list(args)
        if partition_name is not None:
            operands.append(b2j.partition_id_tensor())
        outs = b2j._bass_exec_p.bind(
            *operands,
            out_avals=tuple(out_avals),
            in_names=tuple(all_names),
            out_names=tuple(out_names),
            lowering_input_output_aliases=(),
            sim_require_finite=True,
            sim_require_nnan=True,
            nc=nc,
        )
        return tuple(outs)

    devices = jax.devices()[:NCORES]
    mesh = Mesh(np.asarray(devices), ("core",))
    in_specs = (PartitionSpec("core"),) * (n_params + n_outs)
    out_specs = (PartitionSpec("core"),) * n_outs
    sharded = jax.jit(
        shard_map(_body, mesh=mesh, in_specs=in_specs,
                  out_specs=out_specs, check_rep=False),
        donate_argnums=donate,
        keep_unused=True,
    )
    runner = {
        "jit": sharded, "in_names": in_names, "out_names": out_names,
        "out_avals": out_avals, "zero_shapes": zero_shapes, "mesh": mesh,
    }
    _runner_cache[key] = runner
    return runner


def _run_cores(in_maps):
    runner = _get_runner(1)
    concat_in = [
        np.concatenate([in_maps[c][name] for c in range(NCORES)], axis=0)
        for name in runner["in_names"]
    ]
    concat_zeros = [
        np.zeros((NCORES * s[0], *s[1:]), d) for (s, d) in runner["zero_shapes"]
    ]
    out_arrs = runner["jit"](*concat_in, *concat_zeros)
    results = []
    for c in range(NCORES):
        results.append({
            name: np.asarray(out_arrs[i]).reshape(
                NCORES, *runner["out_avals"][i].shape)[c]
            for i, name in enumerate(runner["out_names"])
        })
    return results


def _pipelined_per_call(runner, in_maps, iters=10):
    """Enqueue `iters` executions back-to-back and block once; the
    per-call slope removes the blocking round-trip latency."""
    import time as _time
    import jax
    from jax.sharding import NamedSharding, PartitionSpec
    mesh = runner["mesh"]
    shard = NamedSharding(mesh, PartitionSpec("core"))
    concat_in = [
        np.concatenate([in_maps[c][name] for c in range(NCORES)], axis=0)
        for name in runner["in_names"]
    ]
    dev_in = [jax.device_put(a, shard) for a in concat_in]
    fn = runner["jit"]

    def zeros_dev():
        return [
            jax.device_put(np.zeros((NCORES * s[0], *s[1:]), d), shard)
            for (s, d) in runner["zero_shapes"]
        ]

    out = fn(*dev_in, *zeros_dev())
    jax.block_until_ready(out)
    first_out = [np.asarray(o) for o in out]
    best = None
    for _rep in range(3):
        zsets = [zeros_dev() for _ in range(iters)]
        jax.block_until_ready(zsets)
        outs = []
        t0 = _time.perf_counter()
        for z in zsets:
            outs.append(fn(*dev_in, *z))
        jax.block_until_ready(outs)
        t1 = _time.perf_counter()
        per_call = (t1 - t0) / iters
        if best is None or per_call < best:
            best = per_call
    return best, first_out


def benchmark(in_maps, iters_hi=256, calls=12):
    """Hardware exec time per kernel instance: the same kernel body is
    repeated iters_hi times via a hardware For_i loop inside one NEFF;
    the marginal cost (T(iters_hi) - T(1)) / (iters_hi - 1) is pure
    device execution, with the (multi-ms) per-call host/axon dispatch
    overhead cancelled."""
    r1 = _get_runner(1)
    rh = _get_runner(iters_hi)
    t1, out1 = _pipelined_per_call(r1, in_maps, iters=calls)
    th, outh = _pipelined_per_call(rh, in_maps, iters=calls)
    if not PROBE:
        for a, b in zip(out1, outh):
            assert np.array_equal(a, b), "looped kernel output mismatch"
    hw = (th - t1) / (iters_hi - 1)
    return hw, t1, th


def _prep_core_inputs(Q, K, sketching_matrix, random_sign):
    """Host-side shard prep: per core one packed [128, INW] array."""
    import ml_dtypes
    X = np.concatenate([np.asarray(Q, np.float32),
                        np.asarray(K, np.float32)], axis=2)  # [B,H,4096,64]
    sk = np.asarray(sketching_matrix).astype(np.int64)       # [B, M, D]
    sign = np.asarray(random_sign, dtype=np.float32)         # [M, D]

    # sign-delta weight matrix W[32m+dl, 32c+dl] = sign[m, 32c+dl]
    W = np.zeros((128, D), dtype=np.float32)
    for m in range(M):
        for c in range(D // 32):
            dl = np.arange(32)
            W[32 * m + dl, 32 * c + dl] = sign[m, 32 * c + dl]

    in_maps = []
    for core in range(NCORES):
        packed = np.zeros((128, INW), dtype=np.float32)
        for pr in range(PAIRS):
            pair = core * PAIRS + pr
            b, h = divmod(pair, H)
            Xp = X[b, h]                            # [4096, 64]
            packed[64 * pr:64 * (pr + 1), 0:SEQ2] = Xp.T
            lm = Xp[sk[b]]                          # [M, D, 64]
            # landmark order l' = 128c + 32m + dl where d = 32c + dl;
            # pair pr's landmark block is zero outside its 64 X-feature
            # rows so the full-contract MM1 drops the other pair's X
            lmp = lm.reshape(M, D // 32, 32, P).transpose(1, 0, 2, 3)
            lmp = lmp.reshape(L, P)                 # [(c, m, dl), 64]
            packed[64 * pr:64 * (pr + 1),
                   SEQ2 + L * pr:SEQ2 + L * (pr + 1)] = lmp.T
        packed[:, SEQ2 + PAIRS * L:INW] = W
        in_maps.append({"inp": packed.astype(ml_dtypes.bfloat16)})
    return in_maps


def kernel(Q, K, sketching_matrix, random_sign):
    in_maps = _prep_core_inputs(Q, K, sketching_matrix, random_sign)
    results = _run_cores(in_maps)
    # unshard: device out [PAIRS, 128, 8192] bf16 (p, h*4096+n) with
    # d = 128h + p  ->  [B, H, 4096, 256] f32
    AS = np.empty((B, H, SEQ2, D), dtype=np.float32)
    for core in range(NCORES):
        o = results[core]["out"]                # [PAIRS, 128, 8192] bf16
        for pr in range(PAIRS):
            pair = core * PAIRS + pr
            b, h = divmod(pair, H)
            op = np.asarray(o[pr]).reshape(128, 2, SEQ2).transpose(1, 0, 2)
            AS[b, h] = op.reshape(D, SEQ2).T.astype(np.float32)
    return AS


# revision 38
# speedup vs baseline: 1.2422x; 1.2422x over previous
"""Trainium2 Bass kernel for the sketched-attention RS_SM op.

Reference semantics (per (b,h) pair):
    X  = concat([Q, K], axis=seq)                      # [4096, 64]
    XS = gather of 1024 landmark rows of X             # [m=4, d=256, 64]
    AS[n, d] = sum_m sign[m, d] * exp(X[n] . XS[m, d]) # [4096, 256]

Sharding: 16 (b,h) pairs over 8 cores = 2 pairs/core, no cross-core comms.

ALL matmuls run in the single 128x32 col-tiled PE mode (4 concurrent
32-wide tiles) -- mixing tiling modes forces a TensorE drain per switch,
which cost ~18us/run here.  Pipeline per (token-chunk t of 512, pair):
  MM1  : lhsT = landmarksT [128, 32] (zero-padded outside the pair's 64
         X-feature rows, so the full-contract matmul over the stacked
         [pair0;pair1] X^T drops the other pair), rhs = X^T [128, 512]
         -> PSUM [32 lmk, 512 n] per col tile, 4 tiles per chunk.
  exp  : split across two engines reading PSUM directly:
         - ScalarE activation Exp for chunks c0..c3 -> SBUF bf16
         - VectorE Schraudolph fast-exp for chunks c4..c7: one
           tensor_scalar (x*A + B) with int16 output whose bits ARE the
           bf16 of exp(x)  (A = 128/ln2, B = 127*128 - C).
  MM2  : lhsT = sign-delta W [128, 32], rhs = exp tile [128, 512]
         -> PSUM [32 d, 512 n], 4 col-tiles concurrent per half.
         This performs the signed reduction over m on the TensorE.
         MM2 is software-pipelined one step behind MM1 so the PE queue
         never blocks on an exp that hasn't finished.
  copy : PSUM -> SBUF bf16 [128, 512] per half, balanced between
         ScalarE and VectorE so both engines finish together.
  out  : two contiguous DMAs per pair: [128, 2, 2048] bf16.

Landmark order is permuted (host-side) so chunk c holds (m, dl) for
d = 32c + dl: partition p = 32*m + dl.  W[32m+dl, 32c+dl] = sign[m, 32c+dl].
Output HBM layout is [pair, 128 p, 2 h, 4096 n] bf16 with d = 128h + p;
the host decodes to [4096, 256] f32 at unshard.

All three device inputs (X^T | landmarks^T | W) are packed into one
[128, 5376] array so a single DMA (one semaphore lane) feeds the PE --
multiple DMA waits on one fused-LDW matmul overflow its sync-wait slots.

KERNEL_ITERS repeats the body (same in/out) inside one NEFF so the
benchmark can measure the marginal per-iteration hardware time,
excluding the multi-ms host->axon dispatch overhead.
"""

import math
import os
import sys
import types
from contextlib import ExitStack

import numpy as np

sys.path.insert(0, "/opt/trn_rl_repo")

# The axon client in this container lacks the NTFF profile hook module;
# provide a stub so bass_utils' trace path degrades gracefully.
try:
    import antenv.axon_hooks  # noqa: F401
except ImportError:
    _stub = types.ModuleType("antenv.axon_hooks")
    _stub.get_axon_ntff_profile_hook = lambda: None
    sys.modules["antenv.axon_hooks"] = _stub

import concourse.bacc as bacc
import concourse.bass as bass
import concourse.mybir as mybir
import concourse.tile as tile

B, H, N, P = 2, 8, 2048, 64
M, D = 4, 256
SEQ2 = 2 * N                      # 4096 tokens per pair
NCORES = 8
PAIRS = (B * H) // NCORES         # 2 pairs per core
L = M * D                         # 1024 landmarks per pair
TCH = 512                         # token chunk (matmul moving dim)
NT = SEQ2 // TCH                  # 8 token chunks
NC_ = 8                           # landmark chunks of 128
INW = SEQ2 + PAIRS * L + D        # packed input width: xt | lt0 | lt1 | w
F32 = mybir.dt.float32
BF16 = mybir.dt.bfloat16
I16 = mybir.dt.int16

# Schraudolph fast-exp in bf16-bit domain: bits16 = round(x*A + B);
# bitcast16 -> bf16 ~= exp(x).  C trades max vs rms error.
EXP_A = 128.0 / math.log(2.0)
EXP_C = float(os.environ.get("KERNEL_EXP_C", "5.0"))
EXP_B = 127.0 * 128.0 - EXP_C

# of the 4 exp groups (2 chunks each), how many run on VectorE via the
# approximate fast-exp (taken from the top: groups 3, 2, ...).
DVE_GROUPS = int(os.environ.get("KERNEL_DVE_GROUPS", "2"))

# timing-only bisection probes: "no_mm1" / "no_mm2" / "no_exp" / "no_dma"
PROBE = os.environ.get("KERNEL_PROBE", "")

_nc_cache = {}


def _copy_split(dve_groups: int) -> int:
    """# of the 32 PSUM->SBUF output copies to run on ScalarE so that
    ScalarE (exp + copies) and VectorE (exp + copies) finish together.
    Constants are CoreSim-calibrated per-instruction durations (ns)."""
    # HW-measured ScalarE overhead is ~352 cyc/instr (not the errata 172),
    # so weight the split with the (N+352)/1.2 and (N+151)/0.96 formulas.
    ov = os.environ.get("KERNEL_SC_COPIES")
    if ov is not None:
        return max(0, min(32, int(ov)))
    sc_exp = 16 * (4 - dve_groups) * 1147.0
    ve_exp = 16 * dve_groups * 1224.0
    # sc_exp + x*720 == ve_exp + (32-x)*691
    x = (ve_exp + 32 * 691.0 - sc_exp) / (720.0 + 691.0)
    return max(0, min(32, int(round(x))))


def _build_nc(iters: int = 1):
    nc = bacc.Bacc(
        "TRN2", target_bir_lowering=False, debug=False, num_devices=NCORES,
    )

    inp = nc.dram_tensor("inp", [128, INW], BF16, kind="ExternalInput")
    out = nc.dram_tensor("out", [PAIRS, 128, 2 * SEQ2], BF16,
                         kind="ExternalOutput")

    n_scalar_copies = _copy_split(DVE_GROUPS)

    with tile.TileContext(nc) as tc, ExitStack() as ctx:
        inp_pool = ctx.enter_context(tc.tile_pool(name="inp", bufs=2))
        eps_pool = ctx.enter_context(
            tc.tile_pool(name="eps", bufs=3, space="PSUM"))
        asps_pool = ctx.enter_context(
            tc.tile_pool(name="asps", bufs=2, space="PSUM"))
        esb_pool = ctx.enter_context(tc.tile_pool(name="esb", bufs=2))
        assb_pool = ctx.enter_context(tc.tile_pool(name="assb", bufs=2))

        def body():
            inp_sb = inp_pool.tile([128, INW], BF16, tag="inp")
            nc.sync.dma_start(inp_sb[:], inp[:])
            xt_sb = inp_sb[:, 0:SEQ2]
            lt_sb = [inp_sb[:, SEQ2 + L * p:SEQ2 + L * (p + 1)]
                     for p in range(PAIRS)]
            w_sb = inp_sb[:, SEQ2 + PAIRS * L:INW]

            # software pipeline over 16 steps s = (pr, t): step s emits
            # MM1+exp for s and MM2+copy for s-1, so the PE never sits in
            # its queue behind an MM2 whose exp hasn't finished.
            state = {}          # step -> (e_sb, as_sb, pr, t)
            as_tiles = {}

            def stage1(s):
                t, pr = divmod(s, PAIRS)
                if t == 0:
                    as_tiles[pr] = assb_pool.tile(
                        [128, 2 * SEQ2], BF16, tag="assb", name="as_sb")
                # full-contract rhs: both pairs' X stacked; the other
                # pair's rows in lhsT are zero, killing cross terms
                rhs_x = xt_sb[:, t * TCH:(t + 1) * TCH]
                e_sb = esb_pool.tile([128, NC_ * TCH], BF16, tag="esb")
                for g in range(4):
                    e_ps = eps_pool.tile([128, 2 * TCH], F32, tag="eps")
                    for gi in range(2):
                        c = 2 * g + gi
                        if PROBE == "no_mm1" and gi > 0:
                            continue
                        # uniform 128x32 PE tiling mode: 4 concurrent col
                        # tiles, one per 32-landmark (m) slice of chunk c;
                        # contract rows come from the operands' partitions
                        for k in range(4):
                            nc.tensor.matmul(
                                e_ps[32 * k:32 * (k + 1),
                                     gi * TCH:(gi + 1) * TCH],
                                lhsT=lt_sb[pr][:,
                                              128 * c + 32 * k:
                                              128 * c + 32 * (k + 1)],
                                rhs=rhs_x,
                                start=True, stop=True,
                                tile_position=(0, 32 * k),
                            )
                    dst = e_sb[:, 2 * g * TCH:2 * (g + 1) * TCH]
                    if PROBE == "no_exp":
                        continue
                    if g >= 4 - DVE_GROUPS:
                        # VectorE fast-exp: bf16 bits via int16 write
                        nc.vector.tensor_scalar(
                            dst.bitcast(I16),
                            e_ps[:],
                            EXP_A, EXP_B,
                            mybir.AluOpType.mult, mybir.AluOpType.add,
                        )
                    else:
                        nc.scalar.activation(
                            dst, e_ps[:],
                            mybir.ActivationFunctionType.Exp,
                        )
                state[s] = (e_sb, as_tiles[pr], pr, t)

            def stage2(s):
                e_sb, as_sb, pr, t = state.pop(s)
                as_hview = as_sb[:].rearrange("p (h n) -> p h n", h=2)
                for half in range(2):
                    as_ps = asps_pool.tile([128, TCH], F32, tag="asps",
                                           name="as_ps")
                    for j in range(4):
                        c = 4 * half + j
                        if PROBE == "no_mm2" and j > 0:
                            continue
                        nc.tensor.matmul(
                            as_ps[32 * j:32 * (j + 1), :],
                            lhsT=w_sb[:, 32 * c:32 * (c + 1)],
                            rhs=e_sb[:, c * TCH:(c + 1) * TCH],
                            start=True, stop=True,
                            tile_position=(0, 32 * j),
                        )
                    # copy PSUM -> SBUF bf16, engine-balanced
                    dst = as_hview[:, half, t * TCH:(t + 1) * TCH]
                    k = 2 * s + half
                    if (k * n_scalar_copies) // 32 != \
                            ((k + 1) * n_scalar_copies) // 32:
                        nc.scalar.copy(dst, as_ps[:])
                    else:
                        nc.vector.tensor_copy(dst, as_ps[:])
                # drain finished token-chunk ranges early: after t=3 and t=7
                if PROBE == "no_dma":
                    return
                if t == NT // 2 - 1:
                    nc.sync.dma_start(
                        out[pr].rearrange("p (h n) -> p h n", h=2)[
                            :, :, 0:(NT // 2) * TCH],
                        as_hview[:, :, 0:(NT // 2) * TCH])
                elif t == NT - 1:
                    nc.sync.dma_start(
                        out[pr].rearrange("p (h n) -> p h n", h=2)[
                            :, :, (NT // 2) * TCH:SEQ2],
                        as_hview[:, :, (NT // 2) * TCH:SEQ2])

            for s in range(PAIRS * NT):
                stage1(s)
                if s > 0:
                    stage2(s - 1)
            stage2(PAIRS * NT - 1)

        if iters == 1:
            body()
        else:
            # unroll 4 bodies per hardware-loop iteration so the per-
            # iteration all-engine barrier amortizes over 4 kernel runs
            assert iters % 4 == 0, iters
            with tc.For_i(0, iters // 4):
                for _u in range(4):
                    body()
    nc.compile()
    return nc


def _get_nc(iters: int = 1):
    key = (iters, DVE_GROUPS)
    if key not in _nc_cache:
        _nc_cache[key] = _build_nc(iters)
    return _nc_cache[key]


_runner_cache = {}


def _get_runner(iters: int = 1):
    """Build (once) a jitted shard_map callable over the 8 cores, mirroring
    bass2jax.run_bass_via_pjrt but cached so repeat calls don't re-trace."""
    key = (iters, DVE_GROUPS)
    if key in _runner_cache:
        return _runner_cache[key]
    import jax
    from jax.sharding import Mesh, PartitionSpec
    try:
        from jax.experimental.shard_map import shard_map
    except ImportError:
        from jax.shard_map import shard_map  # newer jax
    from concourse import bass2jax as b2j

    b2j.install_neuronx_cc_hook()
    nc = _get_nc(iters)

    partition_name = (
        nc.partition_id_tensor.name if nc.partition_id_tensor else None
    )
    in_names, out_names, out_avals, zero_shapes = [], [], [], []
    for alloc in nc.m.functions[0].allocations:
        if not isinstance(alloc, mybir.MemoryLocationSet):
            continue
        name = alloc.memorylocations[0].name
        if alloc.kind == "ExternalInput":
            if name != partition_name:
                in_names.append(name)
        elif alloc.kind == "ExternalOutput":
            out_names.append(name)
            shape = tuple(alloc.tensor_shape)
            dtype = mybir.dt.np(alloc.dtype)
            out_avals.append(jax.core.ShapedArray(shape, dtype))
            zero_shapes.append((shape, dtype))
    n_params = len(in_names)
    n_outs = len(out_avals)
    all_names = list(in_names) + list(out_names)
    if partition_name is not None:
        all_names.append(partition_name)
    donate = tuple(range(n_params, n_params + n_outs))

    def _body(*args):
        operands = list(args)
        if partition_name is not None:
            operands.append(b2j.partition_id_tensor())
        outs = b2j._bass_exec_p.bind(
            *operands,
            out_avals=tuple(out_avals),
            in_names=tuple(all_names),
            out_names=tuple(out_names),
            lowering_input_output_aliases=(),
            sim_require_finite=True,
            sim_require_nnan=True,
            nc=nc,
        )
        return tuple(outs)

    devices = jax.devices()[:NCORES]
    mesh = Mesh(np.asarray(devices), ("core",))
    in_specs = (PartitionSpec("core"),) * (n_params + n_outs)
    out_specs = (PartitionSpec("core"),) * n_outs
    sharded = jax.jit(
        shard_map(_body, mesh=mesh, in_specs=in_specs,
                  out_specs=out_specs, check_rep=False),
        donate_argnums=donate,
        keep_unused=True,
    )
    runner = {
        "jit": sharded, "in_names": in_names, "out_names": out_names,
        "out_avals": out_avals, "zero_shapes": zero_shapes, "mesh": mesh,
    }
    _runner_cache[key] = runner
    return runner


def _run_cores(in_maps):
    runner = _get_runner(1)
    concat_in = [
        np.concatenate([in_maps[c][name] for c in range(NCORES)], axis=0)
        for name in runner["in_names"]
    ]
    concat_zeros = [
        np.zeros((NCORES * s[0], *s[1:]), d) for (s, d) in runner["zero_shapes"]
    ]
    out_arrs = runner["jit"](*concat_in, *concat_zeros)
    results = []
    for c in range(NCORES):
        results.append({
            name: np.asarray(out_arrs[i]).reshape(
                NCORES, *runner["out_avals"][i].shape)[c]
            for i, name in enumerate(runner["out_names"])
        })
    return results


def _pipelined_per_call(runner, in_maps, iters=10):
    """Enqueue `iters` executions back-to-back and block once; the
    per-call slope removes the blocking round-trip latency."""
    import time as _time
    import jax
    from jax.sharding import NamedSharding, PartitionSpec
    mesh = runner["mesh"]
    shard = NamedSharding(mesh, PartitionSpec("core"))
    concat_in = [
        np.concatenate([in_maps[c][name] for c in range(NCORES)], axis=0)
        for name in runner["in_names"]
    ]
    dev_in = [jax.device_put(a, shard) for a in concat_in]
    fn = runner["jit"]

    def zeros_dev():
        return [
            jax.device_put(np.zeros((NCORES * s[0], *s[1:]), d), shard)
            for (s, d) in runner["zero_shapes"]
        ]

    out = fn(*dev_in, *zeros_dev())
    jax.block_until_ready(out)
    first_out = [np.asarray(o) for o in out]
    best = None
    for _rep in range(3):
        zsets = [zeros_dev() for _ in range(iters)]
        jax.block_until_ready(zsets)
        outs = []
        t0 = _time.perf_counter()
        for z in zsets:
            outs.append(fn(*dev_in, *z))
        jax.block_until_ready(outs)
        t1 = _time.perf_counter()
        per_call = (t1 - t0) / iters
        if best is None or per_call < best:
            best = per_call
    return best, first_out


def benchmark(in_maps, iters_lo=64, iters_hi=256, calls=12):
    """Hardware exec time per kernel instance: the same kernel body is
    repeated N times via a hardware For_i loop inside one NEFF; the
    marginal cost (T(iters_hi) - T(iters_lo)) / (iters_hi - iters_lo)
    is pure device execution.  Both ends are loop NEFFs with identical
    dispatch structure, so the (multi-ms, load-varying) host/axon
    dispatch overhead cancels cleanly."""
    rl = _get_runner(iters_lo)
    rh = _get_runner(iters_hi)
    tl, outl = _pipelined_per_call(rl, in_maps, iters=calls)
    th, outh = _pipelined_per_call(rh, in_maps, iters=calls)
    if not PROBE:
        for a, b in zip(outl, outh):
            assert np.array_equal(a, b), "looped kernel output mismatch"
    hw = (th - tl) / (iters_hi - iters_lo)
    return hw, tl, th


def _prep_core_inputs(Q, K, sketching_matrix, random_sign):
    """Host-side shard prep: per core one packed [128, INW] array."""
    import ml_dtypes
    X = np.concatenate([np.asarray(Q, np.float32),
                        np.asarray(K, np.float32)], axis=2)  # [B,H,4096,64]
    sk = np.asarray(sketching_matrix).astype(np.int64)       # [B, M, D]
    sign = np.asarray(random_sign, dtype=np.float32)         # [M, D]

    # sign-delta weight matrix W[32m+dl, 32c+dl] = sign[m, 32c+dl]
    W = np.zeros((128, D), dtype=np.float32)
    for m in range(M):
        for c in range(D // 32):
            dl = np.arange(32)
            W[32 * m + dl, 32 * c + dl] = sign[m, 32 * c + dl]

    in_maps = []
    for core in range(NCORES):
        packed = np.zeros((128, INW), dtype=np.float32)
        for pr in range(PAIRS):
            pair = core * PAIRS + pr
            b, h = divmod(pair, H)
            Xp = X[b, h]                            # [4096, 64]
            packed[64 * pr:64 * (pr + 1), 0:SEQ2] = Xp.T
            lm = Xp[sk[b]]                          # [M, D, 64]
            # landmark order l' = 128c + 32m + dl where d = 32c + dl;
            # pair pr's landmark block is zero outside its 64 X-feature
            # rows so the full-contract MM1 drops the other pair's X
            lmp = lm.reshape(M, D // 32, 32, P).transpose(1, 0, 2, 3)
            lmp = lmp.reshape(L, P)                 # [(c, m, dl), 64]
            packed[64 * pr:64 * (pr + 1),
                   SEQ2 + L * pr:SEQ2 + L * (pr + 1)] = lmp.T
        packed[:, SEQ2 + PAIRS * L:INW] = W
        in_maps.append({"inp": packed.astype(ml_dtypes.bfloat16)})
    return in_maps


def kernel(Q, K, sketching_matrix, random_sign):
    in_maps = _prep_core_inputs(Q, K, sketching_matrix, random_sign)
    results = _run_cores(in_maps)
    # unshard: device out [PAIRS, 128, 8192] bf16 (p, h*4096+n) with
    # d = 128h + p  ->  [B, H, 4096, 256] f32
    AS = np.empty((B, H, SEQ2, D), dtype=np.float32)
    for core in range(NCORES):
        o = results[core]["out"]                # [PAIRS, 128, 8192] bf16
        for pr in range(PAIRS):
            pair = core * PAIRS + pr
            b, h = divmod(pair, H)
            op = np.asarray(o[pr]).reshape(128, 2, SEQ2).transpose(1, 0, 2)
            AS[b, h] = op.reshape(D, SEQ2).T.astype(np.float32)
    return AS
